# revision 31
# baseline (speedup 1.0000x reference)
"""Trainium2 Bass kernel for nn_CapsuleLayer (dynamic routing, 3 iterations).

Math (reference):
    u_hat[b,c,u,s] = sum_i W[c,u,s,i] x[b,i,c]          (B=256,C=1152,U=10,S=16,I=8)
    3x routing:  c_ij = softmax_u(b_ij);  s_j = sum_c c_ij*u_hat;  v = squash(s_j)
                 b_ij += mean_b(u_hat . v)
    return v[..., None]

u_hat is 189MB in fp32 — never materialized.  Both contractions against
u_hat factor through x and W directly:
    s_j[b,us]  = sum_{ci} x[ci,b] * (c_ij[c,u] * W[ci,us])       (PE matmuls)
    a[c,u]     = sum_{s,i} W[ci,us] * G[ci,us],
    G[ci,us]   = sum_b x[b,ci] * v[b,us]                          (PE matmuls)
The W.G elementwise product + s-reduce runs on DVE; the i-reduce (partition
groups of 8) via a tiny selection matmul on PE.  Data-parallel over batch:
each core takes 32 batches; the per-iteration agreement partial [1152,10]
is AllReduce-summed (x1/256 folded into the selection matrix).

K-ordering: k=(c,i), chunk kb holds c in [16kb,16kb+16), partition
p = (c%16)*8 + i.  All heavy tensors are host-pre-transposed so every big
DMA is contiguous.

Configs: "f32" (exact), "f32r" (PE in rounded-fp32 mode), "f32rp" (f32r
with U padded 10->16 so matmul N=256 streams at full rate), "bf16".
"""

import numpy as np

import concourse.bass as bass
import concourse.bacc as bacc
import concourse.tile as tile
from concourse import mybir
from concourse import bass_utils

# ------------------------------------------- custom DVE op: prefix(W*G)
# One DVE pass computes running sums of the elementwise product; segment
# sums (over s) then fall out of a strided subtract of the prefix ends.


def _register_mulscan():
    import numpy as np
    from concourse import dve_ops
    from concourse.dve_spec import Spec, Src0, Src1, AluOp, scan, lower
    from concourse.dve_uop import DveOpSpec

    name = "CAPS_MULSCAN_ANT"
    for op in dve_ops.OPS:
        if op.name == name:
            return op
    spec = Spec(
        body=scan(AluOp.ADD, Src0 * Src1),
        reference=lambda in0, in1, s0, s1, imm2: np.cumsum(
            np.asarray(in0, np.float32).reshape(in0.shape[0], -1)
            * np.asarray(in1, np.float32).reshape(in1.shape[0], -1),
            axis=1,
        ),
    )
    row = dve_ops._CUSTOM_DVE_ROW_BASE + len(dve_ops.OPS)
    shas = {}
    for ver in ("v3", "v4"):
        s = DveOpSpec(name=name, opcode=row, uops=lower(spec, ver=ver), rd1_en=True)
        shas[ver] = s.sha(ver)
    op = dve_ops.DveOp(name, spec, subdim=False, uops_sha=shas)
    dve_ops.OPS.append(op)
    dve_ops.CUSTOM_DVE_SPECS[name] = spec
    dve_ops._SUB_OPCODE_FOR_NAME[name] = row
    return op


MULSCAN = _register_mulscan()


def _pin_act_tables():
    """Make natural_log_exp_and_others the unique candidate set for
    exp/ln/square so bacc's table-load pass never alternates sets.
    Set ids are positional, so entries are filtered in place, never
    reordered."""
    import functools
    import concourse.bacc as _bacc
    import concourse.hw_specs as _hw
    if getattr(_bacc, "_caps_act_pinned", False):
        return
    orig = _hw.get_activation_tables

    @functools.cache
    def pinned(module_arch):
        tables = dict(orig(module_arch))
        keep = "natural_log_exp_and_others"
        assert keep in tables
        only = tables[keep]
        excl = {f for f in only}
        out = {}
        for name, funcs in tables.items():
            if name == keep:
                out[name] = funcs
            else:
                out[name] = funcs - excl
        return out

    _bacc.get_activation_tables = pinned
    _hw.get_activation_tables_orig = orig
    _bacc._caps_act_pinned = True


_pin_act_tables()

# ---------------------------------------------------------------- constants
B, I, C, U, S = 256, 8, 1152, 10, 16
NCORES = 8
BL = B // NCORES            # 32 batches per core
KT = C // 16                # 72 K-chunks of 128 (16 c x 8 i)
EPS = 1e-8
NUM_ROUTING = 3

MM_CFG = "f32rp"

_DT = {
    "f32": mybir.dt.float32,
    "f32r": mybir.dt.float32r,
    "f32rp": mybir.dt.float32r,
    "bf16": mybir.dt.bfloat16,
}


def _np_dt(cfg):
    if cfg == "bf16":
        import ml_dtypes
        return ml_dtypes.bfloat16
    return np.float32


def _up(cfg):
    return 16 if cfg == "f32rp" else U


# ---------------------------------------------------------------- device code
def build_nc(cfg=MM_CFG, repeat=1, collectives=True):
    nc = bacc.Bacc(
        "TRN2",
        target_bir_lowering=False,
        debug=False,
        num_devices=NCORES if collectives else 1,
    )
    mdt = _DT[cfg]
    f32 = mybir.dt.float32
    NUS = U * S                 # 160 real columns per chunk
    WN = 256 if cfg == "f32rp" else NUS   # matmul rhs window width
    PB = 2                      # G chunks per PSUM bank (256-wide slots)
    CPRA = 3 * PB               # agreement chunks per round (3 banks)
    RA = KT // CPRA             # agreement rounds (12)
    ROUNDS = 8                  # s-pass rounds
    CPR = KT // ROUNDS          # 9 chunks per s-round

    w_d = nc.dram_tensor("w_sb", [128, KT * NUS], mdt, kind="ExternalInput")
    w32_d = (
        nc.dram_tensor("w_f32", [128, KT * NUS], f32, kind="ExternalInput")
        if cfg == "bf16"
        else None
    )
    xt_d = nc.dram_tensor("x_t", [128, KT * BL], mdt, kind="ExternalInput")
    # x_b packed 4-up: partitions 32q..32q+31 hold chunks 18q..18q+17
    xb_d = nc.dram_tensor("x_b", [128, (KT // 4) * 128], mdt, kind="ExternalInput")
    rep_d = nc.dram_tensor("rep", [BL, 128], f32, kind="ExternalInput")
    sel_d = nc.dram_tensor("sel", [128, 16], f32, kind="ExternalInput")
    out_d = nc.dram_tensor("v_out", [BL, NUS], f32, kind="ExternalOutput")

    with tile.TileContext(nc) as tc:
        with (
            tc.tile_pool(name="singles", bufs=1) as singles,
            tc.tile_pool(name="weff_p", bufs=3) as weff_p,
            tc.tile_pool(name="prod_p", bufs=3) as prod_p,
            tc.tile_pool(name="rsum_p", bufs=3) as rsum_p,
            tc.tile_pool(name="small", bufs=2) as small,
            tc.tile_pool(name="bsoft", bufs=2) as bsoft,
            tc.tile_pool(name="ps_s", bufs=1, space="PSUM") as ps_s,
            tc.tile_pool(name="ps_g", bufs=2, space="PSUM") as ps_g,
            tc.tile_pool(name="ps_b", bufs=1, space="PSUM") as ps_b,
            tc.tile_pool(name="dram", bufs=2, space="DRAM") as dram,
        ):
            # ---------------- persistent SBUF loads (contiguous DMAs)
            # x_t first: every P1 matmul needs it; W streams in behind it
            x_t = singles.tile([128, KT, BL], mdt)
            nc.sync.dma_start(out=x_t[:], in_=xt_d[:])
            w_sb = singles.tile([128, KT, U, S], mdt)
            for r in range(ROUNDS):
                nc.sync.dma_start(
                    out=w_sb[:, r * CPR:(r + 1) * CPR, :, :],
                    in_=w_d[:, r * CPR * NUS:(r + 1) * CPR * NUS],
                )
            if cfg == "bf16":
                w32_sb = singles.tile([128, KT, U, S], f32)
                for r in range(ROUNDS):
                    nc.sync.dma_start(
                        out=w32_sb[:, r * CPR:(r + 1) * CPR, :, :],
                        in_=w32_d[:, r * CPR * NUS:(r + 1) * CPR * NUS],
                    )
            else:
                w32_sb = w_sb
            x_b = singles.tile([128, (KT // 4) * 128], mdt)
            nc.sync.dma_start(out=x_b[:], in_=xb_d[:])
            rep = singles.tile([BL, 128], f32)
            nc.sync.dma_start(out=rep[:], in_=rep_d[:])
            sel = singles.tile([128, 16], f32)
            nc.sync.dma_start(out=sel[:], in_=sel_d[:])
            eps_sb = singles.tile([BL, 1], f32)
            nc.vector.memset(eps_sb[:], EPS)

            def s_mm(s_ps, kb, flat, nchunks, j, first, last):
                """One windowed s-matmul: rhs = 256-wide window at chunk j of
                a contiguous [128, nchunks*160] buffer (junk beyond col 160
                lands in unused psum columns); the buffer's final chunk falls
                back to an exact 160-wide matmul."""
                n = min(WN, nchunks * NUS - j * NUS)
                nc.tensor.matmul(
                    out=s_ps[:, :n],
                    lhsT=x_t[:, kb, :],
                    rhs=flat[:, j * NUS:j * NUS + n],
                    start=first,
                    stop=last,
                )

            def s_pass(c_exp):
                """s_raw = X^T.Weff accumulated over all 72 chunks -> psum."""
                s_ps = ps_s.tile([BL, WN], f32)
                kb = 0
                for r in range(ROUNDS):
                    if c_exp is None:
                        weff_flat = w_sb[:].rearrange("p k u s -> p (k u s)")
                        nch, base = KT, r * CPR
                    else:
                        ch = c_exp[r // (ROUNDS // 2)]
                        rh = r % (ROUNDS // 2)
                        weff = weff_p.tile([128, CPR, U, S], mdt, tag="weff")
                        eng = nc.gpsimd if r % 3 == 1 else nc.vector
                        eng.tensor_mul(
                            weff[:],
                            w_sb[:, r * CPR:(r + 1) * CPR, :, :],
                            ch[:, rh * CPR:(rh + 1) * CPR, :, None].broadcast_to(
                                [128, CPR, U, S]
                            ),
                        )
                        weff_flat = weff[:].rearrange("p k u s -> p (k u s)")
                        nch, base = CPR, 0
                    for j in range(CPR):
                        s_mm(s_ps, kb, weff_flat, nch, base + j,
                             kb == 0, kb == KT - 1)
                        kb += 1
                return s_ps

            def squash(s_ps, alpha):
                """v = squash(alpha * s_raw); returns v tile [BL, WN]."""
                s3 = s_ps[:].rearrange("b (u s) -> b u s", s=S)[:, :U, :]
                s2 = small.tile([BL, U, S], f32, tag="s2")
                nc.scalar.activation(
                    out=s2[:], in_=s3, func=mybir.ActivationFunctionType.Square
                )
                sq = small.tile([BL, U], f32, tag="sq")
                nc.vector.reduce_sum(out=sq[:], in_=s2[:], axis=mybir.AxisListType.X)
                if alpha != 1.0:
                    t = small.tile([BL, U], f32, tag="t")
                    nc.vector.tensor_scalar_mul(t[:], sq[:], alpha * alpha)
                else:
                    t = sq
                # rt = sqrt(t + eps) via exp(0.5*ln(t+eps)) (one ACT table set)
                lnt = small.tile([BL, U], f32, tag="lnt")
                nc.scalar.activation(
                    out=lnt[:], in_=t[:],
                    func=mybir.ActivationFunctionType.Ln, bias=eps_sb[:],
                )
                rt = small.tile([BL, U], f32, tag="rt")
                nc.scalar.activation(
                    out=rt[:], in_=lnt[:],
                    func=mybir.ActivationFunctionType.Exp, scale=0.5,
                )
                dd = small.tile([BL, U], f32, tag="dd")
                nc.vector.scalar_tensor_tensor(
                    out=dd[:], in0=t[:], scalar=1.0, in1=rt[:],
                    op0=mybir.AluOpType.add, op1=mybir.AluOpType.mult,
                )
                g = small.tile([BL, U], f32, tag="g")
                nc.vector.reciprocal(g[:], dd[:])
                # af padded to WN/S rows so v can be written full-width
                nu = WN // S
                af = small.tile([BL, nu], f32, tag="af")
                if nu > U:
                    nc.scalar.mul(
                        out=af[:, U:],
                        in_=eps_sb[:, 0:1].broadcast_to([BL, nu - U]),
                        mul=0.0,
                    )
                nc.vector.scalar_tensor_tensor(
                    out=af[:, :U], in0=t[:], scalar=float(alpha), in1=g[:],
                    op0=mybir.AluOpType.mult, op1=mybir.AluOpType.mult,
                )
                v = small.tile([BL, nu, S], f32, tag="v")
                nc.vector.tensor_mul(
                    v[:],
                    s_ps[:].rearrange("b (u s) -> b u s", s=S),
                    af[:, :, None].broadcast_to([BL, nu, S]),
                )
                return v

            def a_pass(v):
                """Agreement partial -> AllReduce -> expanded logit delta."""
                # replicate v to all 4 row groups: v4[32q+b, :] = v[b, :]
                v4_ps = ps_b.tile([128, WN], f32, tag="b_ps")
                nc.tensor.matmul(
                    out=v4_ps[:],
                    lhsT=rep[:],
                    rhs=v[:].rearrange("b u s -> b (u s)"),
                    start=True,
                    stop=True,
                )
                v4 = small.tile([128, WN], mdt, tag="v4")
                nc.scalar.copy(out=v4[:], in_=v4_ps[:])

                a_exps = []
                HKT = KT // 2
                b_part = None
                for r in range(RA):
                    if r % (RA // 2) == 0:
                        b_part = small.tile(
                            [16, HKT * U], f32, tag=f"b_part{r // (RA // 2)}"
                        )
                    g_ps = ps_g.tile([128, 3, 512], f32, tag="g")
                    for j in range(CPRA):
                        kb = r * CPRA + j
                        q = kb // (KT // 4)
                        kq = kb % (KT // 4)
                        nc.tensor.matmul(
                            out=g_ps[
                                :, j // PB, (j % PB) * 256:(j % PB) * 256 + WN
                            ],
                            lhsT=x_b[32 * q:32 * q + 32, kq * 128:(kq + 1) * 128],
                            rhs=v4[32 * q:32 * q + 32, :],
                            start=True,
                            stop=True,
                            tile_position=(32 * q, 0),
                        )
                    w_round = w32_sb[:, r * CPRA:(r + 1) * CPRA, :, :].rearrange(
                        "p k u s -> p k (u s)"
                    )
                    nseg = CPRA * U
                    if True:
                        # fused product + prefix-sum; chunk slots merge into
                        # one strided dim [[256, CPRA], [1, NUS]]
                        g_v3 = (
                            g_ps[:].rearrange("p a m -> p (a m)")
                            .rearrange("p (k m) -> p k m", m=256)[:, :, :NUS]
                        )
                        pref = prod_p.tile([128, 16 * (nseg + 1)], f32, tag="prod")
                        nc.scalar.mul(out=pref[:, 0:1], in_=sel[:, 0:1], mul=0.0)
                        nc.vector._custom_dve(
                            MULSCAN,
                            out=pref[:, 1:1 + CPRA * NUS],
                            in0=w_round,
                            in1=g_v3,
                        )
                        ends = pref[:, S:S + nseg * S].rearrange(
                            "p (n s) -> p n s", s=S
                        )[:, :, 0]
                        prevs = pref[:, 0:nseg * S].rearrange(
                            "p (n s) -> p n s", s=S
                        )[:, :, 0]
                        rsum = rsum_p.tile([128, nseg], f32, tag="rsum")
                        nc.vector.tensor_sub(rsum[:], ends, prevs)
                        rsum_flat = rsum[:]
                    else:
                        g_view = (
                            g_ps[:, :, :PB * WN]
                            .rearrange("p a (c m) -> p a c m", c=PB)[:, :, :, :NUS]
                        )
                        prod = prod_p.tile([128, CPRA * NUS], f32, tag="prod")
                        nc.vector.tensor_mul(
                            prod[:].rearrange("p (a c m) -> p a c m", a=3, c=PB),
                            w_round.rearrange("p (a c) m -> p a c m", a=3),
                            g_view,
                        )
                        rsum = rsum_p.tile([128, CPRA, U], f32, tag="rsum")
                        nc.vector.reduce_sum(
                            out=rsum[:],
                            in_=prod[:].rearrange(
                                "p (k u s) -> p k u s", k=CPRA, u=U
                            ),
                            axis=mybir.AxisListType.X,
                        )
                        rsum_flat = rsum[:].rearrange("p k u -> p (k u)")
                    b_ps = ps_b.tile([16, CPRA * U], f32, tag="b_ps")
                    nc.tensor.matmul(
                        out=b_ps[:],
                        lhsT=sel[:],
                        rhs=rsum_flat,
                        start=True,
                        stop=True,
                    )
                    rh = r % (RA // 2)
                    nc.scalar.copy(
                        out=b_part[:, rh * CPRA * U:(rh + 1) * CPRA * U],
                        in_=b_ps[:],
                    )
                    if rh == RA // 2 - 1:
                        h = r // (RA // 2)
                        a_in = dram.tile([16, HKT * U], f32, tag=f"a_in{h}")
                        a_out = dram.tile([16, HKT * U], f32, tag=f"a_out{h}")
                        nc.sync.dma_start(out=a_in[:], in_=b_part[:])
                        if collectives:
                            nc.gpsimd.collective_compute(
                                "AllReduce",
                                mybir.AluOpType.add,
                                replica_groups=[list(range(NCORES))],
                                ins=[a_in[:].opt()],
                                outs=[a_out[:].opt()],
                            )
                        else:
                            nc.sync.dma_start(out=a_out[:], in_=a_in[:])
                        a_exp = bsoft.tile([128, HKT, U], f32, tag=f"a_exp{h}")
                        nc.sync.dma_start(
                            out=a_exp[:],
                            in_=a_out[:, None, :].broadcast_to([16, 8, HKT * U]),
                        )
                        a_exps.append(a_exp)
                return a_exps

            def softmax(b_exp, h):
                HKT = KT // 2
                e = bsoft.tile([128, HKT, U], f32, tag=f"e{h}")
                nc.scalar.activation(
                    out=e[:], in_=b_exp[:], func=mybir.ActivationFunctionType.Exp
                )
                se = bsoft.tile([128, HKT], f32, tag=f"se{h}")
                nc.vector.reduce_sum(out=se[:], in_=e[:], axis=mybir.AxisListType.X)
                re = bsoft.tile([128, HKT], f32, tag=f"re{h}")
                nc.vector.reciprocal(re[:], se[:])
                c_exp = bsoft.tile([128, HKT, U], f32, tag=f"c_exp{h}")
                nc.vector.tensor_mul(
                    c_exp[:], e[:], re[:, :, None].broadcast_to([128, HKT, U])
                )
                return c_exp

            # ------------------------------------------------ routing loop
            for _rep in range(repeat):
                b_prev = None
                c_exp = None
                v = None
                for it in range(NUM_ROUTING):
                    alpha = 1.0 / U if it == 0 else 1.0
                    s_ps = s_pass(c_exp)
                    v = squash(s_ps, alpha)
                    if it == NUM_ROUTING - 1:
                        break
                    a_exps = a_pass(v)
                    if b_prev is None:
                        b_halves = a_exps
                    else:
                        b_halves = []
                        for h in range(2):
                            bh = bsoft.tile(
                                [128, KT // 2, U], f32, tag=f"b_exp{h}"
                            )
                            nc.vector.tensor_add(
                                bh[:], b_prev[h][:], a_exps[h][:]
                            )
                            b_halves.append(bh)
                    b_prev = b_halves
                    c_exp = [softmax(b_halves[h], h) for h in range(2)]

                nc.sync.dma_start(
                    out=out_d[:],
                    in_=v[:, :U, :].rearrange("b u s -> b (u s)"),
                )

    nc.compile()
    return nc


# ---------------------------------------------------------------- host prep
def prep_inputs(x, weight, cfg=MM_CFG):
    """Full inputs -> per-core in_maps with kernel-ready layouts."""
    x = np.asarray(x, dtype=np.float32)
    weight = np.asarray(weight, dtype=np.float32)
    npdt = _np_dt(cfg)

    # W: [C,U,S,I] -> [128, KT, U, S] with p = (c%16)*8 + i
    w = (
        weight.reshape(KT, 16, U, S, I)
        .transpose(1, 4, 0, 2, 3)          # [16, I, KT, U, S]
        .reshape(128, KT * U * S)
    )
    w_mm = np.ascontiguousarray(w, dtype=npdt)
    sel = np.zeros((128, 16), np.float32)
    sel[np.arange(128), np.arange(128) // 8] = 1.0 / B
    rep = np.zeros((BL, 128), np.float32)
    rep[np.arange(128) % BL, np.arange(128)] = 1.0

    in_maps = []
    for k in range(NCORES):
        xs = x[k * BL:(k + 1) * BL]                      # [BL, I, C]
        xcib = xs.transpose(2, 1, 0).reshape(KT, 16, I, BL)  # c-major
        x_t = (
            xcib.reshape(KT, 128, BL).transpose(1, 0, 2).reshape(128, KT * BL)
        )
        x_b = xs.transpose(0, 2, 1).reshape(BL, KT * 128)    # [BL, (c,i)]
        # pack 4-up: partitions 32q..32q+31 hold chunks 18q..18q+17
        x_b4 = (
            x_b.reshape(BL, 4, (KT // 4) * 128)
            .transpose(1, 0, 2)
            .reshape(128, (KT // 4) * 128)
        )
        m = {
            "w_sb": w_mm,
            "x_t": np.ascontiguousarray(x_t, dtype=npdt),
            "x_b": np.ascontiguousarray(x_b4, dtype=npdt),
            "rep": rep,
            "sel": sel,
        }
        if cfg == "bf16":
            m["w_f32"] = np.ascontiguousarray(w, dtype=np.float32)
        in_maps.append(m)
    return in_maps


def assemble_output(results):
    out = np.empty((B, U, S, 1), np.float32)
    for k in range(NCORES):
        out[k * BL:(k + 1) * BL] = (
            results[k]["v_out"].astype(np.float32).reshape(BL, U, S, 1)
        )
    return out


_NC_CACHE = {}


def _get_nc(cfg=MM_CFG):
    if cfg not in _NC_CACHE:
        _NC_CACHE[cfg] = build_nc(cfg)
    return _NC_CACHE[cfg]


def kernel(x, weight):
    nc = _get_nc()
    in_maps = prep_inputs(x, weight)
    res = bass_utils.run_bass_kernel_spmd(
        nc, in_maps, core_ids=list(range(NCORES))
    )
    return assemble_output(res.results)
